# revision 27
# baseline (speedup 1.0000x reference)
"""Trainium2 Bass kernel for nn_GraphPatchEmbed (patch-embed conv + GCN layer).

Math: the whole module is linear in x.
  feats = patches(x) @ Wc.T            (2x2/stride-2 conv == per-patch matmul, K=12)
  xw    = feats @ gcn_w                -> xw = patches @ (Wc.T @ gcn_w) = P @ Wcomb
  out   = D^-1/2 (A+I') D^-1/2 xw + b  (graph aggregation; edges only touch batch 0)
Aggregation (node axis) and matmul (channel axis) commute, so the stencil is applied
on the host to the 12-row patch tensor, the bias folds in as a 13th all-ones row,
and the device kernel is one memory-bound matmul per core:
  [32768, 13] @ [13, 192]   (8-way row-sharded over B*N)

Device design (v5, emb-major / W-stationary):
  - The bottleneck is PSUM evacuation: only DVE and ACT can read PSUM
    (~1 elem/cycle/partition each; GpSimd has no PSUM port, DMA has no
    PSUM route), and each copy instruction pays a flat PSUM/SBUF access
    penalty. So the goal is full 512-wide PSUM banks and long contiguous
    copies.
  - Stationary = W columns ([13,128] for emb 0:128, [13,64] for emb
    128:192), moving = q [13, 512 nodes] -> each matmul fills one whole
    2KB PSUM bank with a single accumulation group.
  - The PE streams ~0.83 ns/col per row-band (MID p-state, never
    ramps), but matmuls in DIFFERENT row quadrants stream fully
    concurrently (measured: 2 bands 0.42 ns/col, 3 bands 0.28). W and
    q live at partition bases {0, 32, 64}, chunk c in band c%3, so
    consecutive matmuls always overlap and the PE (~18us) stays well
    under the copy bound (~28us).
  - Phase 2 (emb 128:192, M=64) packs two node-chunks per bank at
    output partition bases 0 and 64 so copies always span 128
    partitions.
  - PSUM is one flat [128, 8*512] tile; matmuls and 2-bank copies
    rotate through it, relying on Tile's subtile dependency tracking.
  - Copies alternate DVE (CAST) / ACT (COPY) with a slight ACT bias
    (ACT is 0.83 ns/elem vs DVE 1.04).
  - W rides in the first 192 columns of the q tensor so one DMA primes
    both W and the first node chunks; the q load ramps in chunks on the
    sync queue ahead of all output DMAs.
  - Output fp8e3 with a x4 pre-scale folded into W (host decodes).
"""

import numpy as np

from concourse import bacc, mybir, tile
import concourse.bass as bass
from concourse.bass_utils import run_bass_kernel_spmd

B, CIN, HIMG, WIMG = 4, 3, 512, 512
HG, WG = 256, 256          # grid after 2x2/stride-2 patching
N = HG * WG                # 65536 nodes per image
BN = B * N                 # 262144 total rows
EMB = 192
K = 13                     # 12 patch dims + 1 bias row
NCORES = 8
ROWS = BN // NCORES        # 32768 rows per core
FP8_SCALE = 4.0            # folded into W before the e3m4 downcast

CHUNK = 512                # node-cols per matmul == one full psum bank
NCHUNK = ROWS // CHUNK     # 64 chunks per core
WCOLS = EMB                # W header columns of each strip
NBAND = 3
BASES = (0, 32, 64)        # PE row-band per chunk: chunk c -> BASES[c % 3]
LCH = [(NCHUNK + NBAND - 1 - s) // NBAND for s in range(NBAND)]  # chunks/strip
SCOLS = WCOLS + max(LCH) * CHUNK   # 11456 columns per strip
QCOLS = NBAND * SCOLS              # dram q: [strip0 | strip1 | strip2]

NBANK = 8                  # psum banks; ring of 4 groups x 2 banks
GROUP = 2 * CHUNK          # elems per copy (2 banks)
NR1 = NCHUNK // 2          # 32 phase-1 rounds (2 chunks/round)
NR2 = NCHUNK // 4          # 16 phase-2 rounds (4 chunks/round)
NROUND = NR1 + NR2         # 48 rounds; round r -> banks (2*(r%4), +1)
OCOLS = NROUND * GROUP     # 49152 output cols
SGRP = 2                   # rounds per staging tile / output DMA

# input ramp, per strip: first SYNC_RAMP dispatches go on the sync queue
# (earliest), the rest on the otherwise-idle gpsimd SWDGE queue.
SYNC_RAMP = [WCOLS + 512, 1024]
GP_RAMP = [2048, 4096, 3584]
assert sum(SYNC_RAMP) + sum(GP_RAMP) == SCOLS

# copy-engine assignment per round: 0 -> DVE, 1 -> ACT. ACT is slightly
# faster per element, so it takes 25 of 48.
ENG_OF = [(1 if r % 2 else 0) for r in range(NROUND)]
ENG_OF[24] = 1

_NC_CACHE = {}


def _build_nc(out_bufs=8, eng_of=None):
    eng_of = list(eng_of) if eng_of is not None else list(ENG_OF)
    key = (out_bufs, tuple(eng_of))
    if key in _NC_CACHE:
        return _NC_CACHE[key]
    nc = bacc.Bacc(
        "TRN2",
        target_bir_lowering=False,
        debug=False,
        enable_asserts=False,
        num_devices=NCORES,
        enable_partition_id=False,
    )
    f16 = mybir.dt.float16
    f32 = mybir.dt.float32
    f8 = mybir.dt.float8e3
    q = nc.dram_tensor("q", [K, QCOLS], f16, kind="ExternalInput").ap()
    o8 = nc.dram_tensor("o8", [128, OCOLS], f8, kind="ExternalOutput").ap()

    with tile.TileContext(nc) as tc:
        with (
            tc.tile_pool(name="qp", bufs=1) as qpool,
            tc.tile_pool(name="ps", bufs=1, space=bass.MemorySpace.PSUM) as pspool,
            tc.tile_pool(name="ot", bufs=out_bufs) as opool,
        ):
            qw = qpool.tile([BASES[-1] + K, SCOLS], f16)
            # strip s (chunks with c%3==s) lives at partition base BASES[s];
            # dispatch size-major so round-0 data lands first
            for s, base in enumerate(BASES):
                off = 0
                for csz in SYNC_RAMP:
                    nc.sync.dma_start(
                        out=qw[base:base + K, off:off + csz],
                        in_=q[:, s * SCOLS + off:s * SCOLS + off + csz])
                    off += csz
            offs = [sum(SYNC_RAMP)] * NBAND
            for csz in GP_RAMP:
                for s, base in enumerate(BASES):
                    off = offs[s]
                    nc.gpsimd.dma_start(
                        out=qw[base:base + K, off:off + csz],
                        in_=q[:, s * SCOLS + off:s * SCOLS + off + csz])
                    offs[s] += csz

            ps = pspool.tile([128, NBANK * CHUNK], f32)

            def mov(c):
                """moving AP for node-chunk c (band c%3, local index c//3)."""
                b = BASES[c % NBAND]
                lo = WCOLS + (c // NBAND) * CHUNK
                return qw[b:b + K, lo:lo + CHUNK]

            def w1(c):
                b = BASES[c % NBAND]
                return qw[b:b + K, 0:128]

            def w2(c):
                b = BASES[c % NBAND]
                return qw[b:b + K, 128:EMB]

            ssched = [SGRP] * ((NROUND - 2) // SGRP) + [1, 1]
            s0 = 0
            for snr in ssched:
                ot = opool.tile([128, snr * GROUP], f8)
                for r in range(s0, s0 + snr):
                    poff = (r % 4) * GROUP
                    if r < NR1:
                        for kk in range(2):
                            c = 2 * r + kk
                            nc.tensor.matmul(
                                ps[:, poff + kk * CHUNK:poff + (kk + 1) * CHUNK],
                                w1(c), mov(c), start=True, stop=True,
                            )
                    else:
                        for kk in range(2):
                            c0 = 4 * (r - NR1) + 2 * kk
                            dst = ps[:, poff + kk * CHUNK:poff + (kk + 1) * CHUNK]
                            nc.tensor.matmul(
                                dst[0:64, :], w2(c0), mov(c0),
                                start=True, stop=True,
                            )
                            nc.tensor.matmul(
                                dst[64:128, :], w2(c0 + 1), mov(c0 + 1),
                                start=True, stop=True,
                            )
                    src = ps[:, poff:poff + GROUP]
                    dst = ot[:, (r - s0) * GROUP:(r - s0 + 1) * GROUP]
                    if r < 2 or r == NROUND - 1:
                        # pipeline head/tail: split the round across both
                        # engines (1-bank copies) to start/drain faster
                        nc.vector.tensor_copy(dst[:, 0:CHUNK], src[:, 0:CHUNK])
                        nc.scalar.copy(dst[:, CHUNK:GROUP], src[:, CHUNK:GROUP])
                    elif eng_of[r]:
                        nc.scalar.copy(dst, src)
                    else:
                        nc.vector.tensor_copy(dst, src)
                nc.sync.dma_start(
                    out=o8[:, s0 * GROUP:(s0 + snr) * GROUP], in_=ot[:])
                s0 += snr
    nc.compile()
    _NC_CACHE[key] = nc
    return nc


def _host_prep(x, conv_w, gcn_w, gcn_b):
    x = np.asarray(x, dtype=np.float32)
    conv_w = np.asarray(conv_w, dtype=np.float32)
    gcn_w = np.asarray(gcn_w, dtype=np.float32)
    gcn_b = np.asarray(gcn_b, dtype=np.float32)

    # patches P[b, k, n]: k = (cin, ki, kj), n = r*WG + c
    P = np.ascontiguousarray(
        x.reshape(B, CIN, HG, 2, WG, 2).transpose(0, 1, 3, 5, 2, 4)
    ).reshape(B, 12, N)

    # degrees with self-loops; grid edges exist only for batch 0
    nbr = np.full((HG, WG), 4.0, np.float32)
    nbr[0, :] -= 1; nbr[-1, :] -= 1; nbr[:, 0] -= 1; nbr[:, -1] -= 1
    deg = nbr + 1.0
    deg[HG - 2, WG - 2] += 1.0          # the module's trailing extra edge
    dr = (1.0 / np.sqrt(deg)).ravel()    # dinv per node

    # batch-0 aggregation applied to the patch rows (commutes with the matmul)
    z = (dr[None, :] * P[0]).reshape(12, HG, WG)
    s = z.copy()                          # self-loop term
    s[:, 1:, :] += z[:, :-1, :]
    s[:, :-1, :] += z[:, 1:, :]
    s[:, :, 1:] += z[:, :, :-1]
    s[:, :, :-1] += z[:, :, 1:]
    s[:, HG - 2, WG - 2] += z[:, HG - 1, WG - 1]
    Q0 = dr[None, :] * s.reshape(12, N)

    Q = np.empty((K, BN), np.float32)
    Q[:12, :N] = Q0
    Q[:12, N:] = P[1:].transpose(1, 0, 2).reshape(12, 3 * N)
    Q[12, :] = 1.0                        # bias row

    Wcomb = (conv_w.reshape(EMB, 12).astype(np.float64).T
             @ gcn_w.astype(np.float64)).astype(np.float32)
    Wfull = np.concatenate([Wcomb, gcn_b[None, :]], axis=0)  # (13, 192)
    return Q, Wfull


def _decode_core(o8core, inv):
    """[128, OCOLS] fp8 -> [ROWS, EMB] fp32."""
    o = o8core.astype(np.float32) * inv
    out = np.empty((ROWS, EMB), np.float32)
    # phase 1: cols 0:ROWS, partition p = emb p, col c*512+j = node
    out[:, 0:128] = o[:, :ROWS].reshape(128, NCHUNK, CHUNK) \
        .transpose(1, 2, 0).reshape(ROWS, 128)
    # phase 2: 32 chunks of 512; partitions 0:64 = even node chunks,
    # 64:128 = odd node chunks, emb 128+p
    o2 = o[:, ROWS:].reshape(128, NCHUNK // 2, CHUNK)
    v = out[:, 128:EMB].reshape(NCHUNK // 2, 2, CHUNK, 64)
    v[:, 0] = o2[0:64].transpose(1, 2, 0)
    v[:, 1] = o2[64:128].transpose(1, 2, 0)
    return out


def kernel(x, conv_w, gcn_w, gcn_b, _trace=False, _nc_kwargs=None):
    Q, Wfull = _host_prep(x, conv_w, gcn_w, gcn_b)
    nc = _build_nc(**(_nc_kwargs or {}))
    W16 = (Wfull * FP8_SCALE).astype(np.float16)
    Q16 = Q.astype(np.float16)
    in_maps = []
    for c in range(NCORES):
        qc = Q16[:, c * ROWS:(c + 1) * ROWS].reshape(K, NCHUNK, CHUNK)
        qd = np.zeros((K, NBAND, SCOLS), np.float16)
        qd[:, :, :WCOLS] = W16[:, None, :]
        for s in range(NBAND):
            qd[:, s, WCOLS:WCOLS + LCH[s] * CHUNK] = \
                qc[:, s::NBAND].reshape(K, LCH[s] * CHUNK)
        in_maps.append({"q": qd.reshape(K, QCOLS)})
    res = run_bass_kernel_spmd(nc, in_maps, list(range(NCORES)), trace=_trace)
    inv = np.float32(1.0 / FP8_SCALE)
    out = np.empty((NCORES, ROWS, EMB), np.float32)
    for c in range(NCORES):
        out[c] = _decode_core(res.results[c]["o8"], inv)
    out = out.reshape(B, N, EMB)
    if _trace:
        return out, res
    return out


# revision 28
# speedup vs baseline: 1.0000x; 1.0000x over previous
"""Trainium2 Bass kernel for nn_GraphPatchEmbed (patch-embed conv + GCN layer).

Math: the whole module is linear in x.
  feats = patches(x) @ Wc.T            (2x2/stride-2 conv == per-patch matmul, K=12)
  xw    = feats @ gcn_w                -> xw = patches @ (Wc.T @ gcn_w) = P @ Wcomb
  out   = D^-1/2 (A+I') D^-1/2 xw + b  (graph aggregation; edges only touch batch 0)
Aggregation (node axis) and matmul (channel axis) commute, so the stencil is applied
on the host to the 12-row patch tensor, the bias folds in as a 13th all-ones row,
and the device kernel is one memory-bound matmul per core:
  [32768, 13] @ [13, 192]   (8-way row-sharded over B*N)

Device design (v5, emb-major / W-stationary):
  - The bottleneck is PSUM evacuation: only DVE and ACT can read PSUM
    (~1 elem/cycle/partition each; GpSimd has no PSUM port, DMA has no
    PSUM route), and each copy instruction pays a flat PSUM/SBUF access
    penalty. So the goal is full 512-wide PSUM banks and long contiguous
    copies.
  - Stationary = W columns ([13,128] for emb 0:128, [13,64] for emb
    128:192), moving = q [13, 512 nodes] -> each matmul fills one whole
    2KB PSUM bank with a single accumulation group.
  - The PE streams ~0.83 ns/col per row-band (MID p-state, never
    ramps), but matmuls in DIFFERENT row quadrants stream fully
    concurrently (measured: 2 bands 0.42 ns/col, 3 bands 0.28). W and
    q live at partition bases {0, 32, 64}, chunk c in band c%3, so
    consecutive matmuls always overlap and the PE (~18us) stays well
    under the copy bound (~28us).
  - Phase 2 (emb 128:192, M=64) packs two node-chunks per bank at
    output partition bases 0 and 64 so copies always span 128
    partitions.
  - PSUM is one flat [128, 8*512] tile; matmuls and 2-bank copies
    rotate through it, relying on Tile's subtile dependency tracking.
  - Copies alternate DVE (CAST) / ACT (COPY) with a slight ACT bias
    (ACT is 0.83 ns/elem vs DVE 1.04).
  - W rides in the first 192 columns of the q tensor so one DMA primes
    both W and the first node chunks; the q load ramps in chunks on the
    sync queue ahead of all output DMAs.
  - Output fp8e3 with a x4 pre-scale folded into W (host decodes).
"""

import numpy as np

from concourse import bacc, mybir, tile
import concourse.bass as bass
from concourse.bass_utils import run_bass_kernel_spmd

B, CIN, HIMG, WIMG = 4, 3, 512, 512
HG, WG = 256, 256          # grid after 2x2/stride-2 patching
N = HG * WG                # 65536 nodes per image
BN = B * N                 # 262144 total rows
EMB = 192
K = 13                     # 12 patch dims + 1 bias row
NCORES = 8
ROWS = BN // NCORES        # 32768 rows per core
FP8_SCALE = 4.0            # folded into W before the e3m4 downcast

CHUNK = 512                # node-cols per matmul == one full psum bank
NCHUNK = ROWS // CHUNK     # 64 chunks per core
WCOLS = EMB                # W header columns of each strip
NBAND = 3
BASES = (0, 32, 64)        # PE row-band per chunk: chunk c -> BASES[c % 3]
LCH = [(NCHUNK + NBAND - 1 - s) // NBAND for s in range(NBAND)]  # chunks/strip
SCOLS = WCOLS + max(LCH) * CHUNK   # 11456 columns per strip
QCOLS = NBAND * SCOLS              # dram q: [strip0 | strip1 | strip2]

NBANK = 8                  # psum banks; ring of 4 groups x 2 banks
GROUP = 2 * CHUNK          # elems per copy (2 banks)
NR1 = NCHUNK // 2          # 32 phase-1 rounds (2 chunks/round)
NR2 = NCHUNK // 4          # 16 phase-2 rounds (4 chunks/round)
NROUND = NR1 + NR2         # 48 rounds; round r -> banks (2*(r%4), +1)
OCOLS = NROUND * GROUP     # 49152 output cols
SGRP = 2                   # rounds per staging tile / output DMA

# input ramp, per strip: first SYNC_RAMP dispatches go on the sync queue
# (earliest), the rest on the otherwise-idle gpsimd SWDGE queue.
SYNC_RAMP = [WCOLS + 512, 1024]
GP_RAMP = [2048, 4096, 3584]
assert sum(SYNC_RAMP) + sum(GP_RAMP) == SCOLS

# copy-engine assignment per round: 0 -> DVE, 1 -> ACT. ACT is slightly
# faster per element, so it takes 25 of 48.
ENG_OF = [(1 if r % 2 else 0) for r in range(NROUND)]
ENG_OF[24] = 1

_NC_CACHE = {}


def _build_nc(out_bufs=8, eng_of=None):
    eng_of = list(eng_of) if eng_of is not None else list(ENG_OF)
    key = (out_bufs, tuple(eng_of))
    if key in _NC_CACHE:
        return _NC_CACHE[key]
    nc = bacc.Bacc(
        "TRN2",
        target_bir_lowering=False,
        debug=False,
        enable_asserts=False,
        num_devices=NCORES,
        enable_partition_id=False,
    )
    f16 = mybir.dt.float16
    f32 = mybir.dt.float32
    f8 = mybir.dt.float8e3
    q = nc.dram_tensor("q", [K, QCOLS], f16, kind="ExternalInput").ap()
    o8 = nc.dram_tensor("o8", [128, OCOLS], f8, kind="ExternalOutput").ap()

    with tile.TileContext(nc) as tc:
        with (
            tc.tile_pool(name="qp", bufs=1) as qpool,
            tc.tile_pool(name="ps", bufs=1, space=bass.MemorySpace.PSUM) as pspool,
            tc.tile_pool(name="ot", bufs=out_bufs) as opool,
        ):
            qw = qpool.tile([BASES[-1] + K, SCOLS], f16)
            # strip s (chunks with c%3==s) lives at partition base BASES[s];
            # dispatch size-major so round-0 data lands first
            # dispatch size-major (s0c0, s1c0, s2c0, s0c1, ...) so chunk
            # arrival order matches round order — the Tile scheduler orders
            # matmuls by predicted data arrival, and a strip-major order
            # makes it batch whole strips, stalling the round-order copies
            offs = [0] * NBAND
            for csz in SYNC_RAMP:
                for s, base in enumerate(BASES):
                    off = offs[s]
                    nc.sync.dma_start(
                        out=qw[base:base + K, off:off + csz],
                        in_=q[:, s * SCOLS + off:s * SCOLS + off + csz])
                    offs[s] += csz
            for csz in GP_RAMP:
                for s, base in enumerate(BASES):
                    off = offs[s]
                    nc.gpsimd.dma_start(
                        out=qw[base:base + K, off:off + csz],
                        in_=q[:, s * SCOLS + off:s * SCOLS + off + csz])
                    offs[s] += csz

            ps = pspool.tile([128, NBANK * CHUNK], f32)

            def mov(c):
                """moving AP for node-chunk c (band c%3, local index c//3)."""
                b = BASES[c % NBAND]
                lo = WCOLS + (c // NBAND) * CHUNK
                return qw[b:b + K, lo:lo + CHUNK]

            def w1(c):
                b = BASES[c % NBAND]
                return qw[b:b + K, 0:128]

            def w2(c):
                b = BASES[c % NBAND]
                return qw[b:b + K, 128:EMB]

            ssched = [SGRP] * ((NROUND - 2) // SGRP) + [1, 1]
            s0 = 0
            for snr in ssched:
                ot = opool.tile([128, snr * GROUP], f8)
                for r in range(s0, s0 + snr):
                    poff = (r % 4) * GROUP
                    if r < NR1:
                        for kk in range(2):
                            c = 2 * r + kk
                            nc.tensor.matmul(
                                ps[:, poff + kk * CHUNK:poff + (kk + 1) * CHUNK],
                                w1(c), mov(c), start=True, stop=True,
                            )
                    else:
                        for kk in range(2):
                            c0 = 4 * (r - NR1) + 2 * kk
                            dst = ps[:, poff + kk * CHUNK:poff + (kk + 1) * CHUNK]
                            nc.tensor.matmul(
                                dst[0:64, :], w2(c0), mov(c0),
                                start=True, stop=True,
                            )
                            nc.tensor.matmul(
                                dst[64:128, :], w2(c0 + 1), mov(c0 + 1),
                                start=True, stop=True,
                            )
                    src = ps[:, poff:poff + GROUP]
                    dst = ot[:, (r - s0) * GROUP:(r - s0 + 1) * GROUP]
                    if r < 2 or r == NROUND - 1:
                        # pipeline head/tail: split the round across both
                        # engines (1-bank copies) to start/drain faster
                        nc.vector.tensor_copy(dst[:, 0:CHUNK], src[:, 0:CHUNK])
                        nc.scalar.copy(dst[:, CHUNK:GROUP], src[:, CHUNK:GROUP])
                    elif eng_of[r]:
                        nc.scalar.copy(dst, src)
                    else:
                        nc.vector.tensor_copy(dst, src)
                nc.sync.dma_start(
                    out=o8[:, s0 * GROUP:(s0 + snr) * GROUP], in_=ot[:])
                s0 += snr
    nc.compile()
    _NC_CACHE[key] = nc
    return nc


def _host_prep(x, conv_w, gcn_w, gcn_b):
    x = np.asarray(x, dtype=np.float32)
    conv_w = np.asarray(conv_w, dtype=np.float32)
    gcn_w = np.asarray(gcn_w, dtype=np.float32)
    gcn_b = np.asarray(gcn_b, dtype=np.float32)

    # patches P[b, k, n]: k = (cin, ki, kj), n = r*WG + c
    P = np.ascontiguousarray(
        x.reshape(B, CIN, HG, 2, WG, 2).transpose(0, 1, 3, 5, 2, 4)
    ).reshape(B, 12, N)

    # degrees with self-loops; grid edges exist only for batch 0
    nbr = np.full((HG, WG), 4.0, np.float32)
    nbr[0, :] -= 1; nbr[-1, :] -= 1; nbr[:, 0] -= 1; nbr[:, -1] -= 1
    deg = nbr + 1.0
    deg[HG - 2, WG - 2] += 1.0          # the module's trailing extra edge
    dr = (1.0 / np.sqrt(deg)).ravel()    # dinv per node

    # batch-0 aggregation applied to the patch rows (commutes with the matmul)
    z = (dr[None, :] * P[0]).reshape(12, HG, WG)
    s = z.copy()                          # self-loop term
    s[:, 1:, :] += z[:, :-1, :]
    s[:, :-1, :] += z[:, 1:, :]
    s[:, :, 1:] += z[:, :, :-1]
    s[:, :, :-1] += z[:, :, 1:]
    s[:, HG - 2, WG - 2] += z[:, HG - 1, WG - 1]
    Q0 = dr[None, :] * s.reshape(12, N)

    Q = np.empty((K, BN), np.float32)
    Q[:12, :N] = Q0
    Q[:12, N:] = P[1:].transpose(1, 0, 2).reshape(12, 3 * N)
    Q[12, :] = 1.0                        # bias row

    Wcomb = (conv_w.reshape(EMB, 12).astype(np.float64).T
             @ gcn_w.astype(np.float64)).astype(np.float32)
    Wfull = np.concatenate([Wcomb, gcn_b[None, :]], axis=0)  # (13, 192)
    return Q, Wfull


def _decode_core(o8core, inv):
    """[128, OCOLS] fp8 -> [ROWS, EMB] fp32."""
    o = o8core.astype(np.float32) * inv
    out = np.empty((ROWS, EMB), np.float32)
    # phase 1: cols 0:ROWS, partition p = emb p, col c*512+j = node
    out[:, 0:128] = o[:, :ROWS].reshape(128, NCHUNK, CHUNK) \
        .transpose(1, 2, 0).reshape(ROWS, 128)
    # phase 2: 32 chunks of 512; partitions 0:64 = even node chunks,
    # 64:128 = odd node chunks, emb 128+p
    o2 = o[:, ROWS:].reshape(128, NCHUNK // 2, CHUNK)
    v = out[:, 128:EMB].reshape(NCHUNK // 2, 2, CHUNK, 64)
    v[:, 0] = o2[0:64].transpose(1, 2, 0)
    v[:, 1] = o2[64:128].transpose(1, 2, 0)
    return out


def kernel(x, conv_w, gcn_w, gcn_b, _trace=False, _nc_kwargs=None):
    Q, Wfull = _host_prep(x, conv_w, gcn_w, gcn_b)
    nc = _build_nc(**(_nc_kwargs or {}))
    W16 = (Wfull * FP8_SCALE).astype(np.float16)
    Q16 = Q.astype(np.float16)
    in_maps = []
    for c in range(NCORES):
        qc = Q16[:, c * ROWS:(c + 1) * ROWS].reshape(K, NCHUNK, CHUNK)
        qd = np.zeros((K, NBAND, SCOLS), np.float16)
        qd[:, :, :WCOLS] = W16[:, None, :]
        for s in range(NBAND):
            qd[:, s, WCOLS:WCOLS + LCH[s] * CHUNK] = \
                qc[:, s::NBAND].reshape(K, LCH[s] * CHUNK)
        in_maps.append({"q": qd.reshape(K, QCOLS)})
    res = run_bass_kernel_spmd(nc, in_maps, list(range(NCORES)), trace=_trace)
    inv = np.float32(1.0 / FP8_SCALE)
    out = np.empty((NCORES, ROWS, EMB), np.float32)
    for c in range(NCORES):
        out[c] = _decode_core(res.results[c]["o8"], inv)
    out = out.reshape(B, N, EMB)
    if _trace:
        return out, res
    return out


# revision 36
# speedup vs baseline: 1.0720x; 1.0719x over previous
"""Trainium2 Bass kernel for nn_GraphPatchEmbed (patch-embed conv + GCN layer).

Math: the whole module is linear in x.
  feats = patches(x) @ Wc.T            (2x2/stride-2 conv == per-patch matmul, K=12)
  xw    = feats @ gcn_w                -> xw = patches @ (Wc.T @ gcn_w) = P @ Wcomb
  out   = D^-1/2 (A+I') D^-1/2 xw + b  (graph aggregation; edges only touch batch 0)
Aggregation (node axis) and matmul (channel axis) commute, so the stencil is applied
on the host to the 12-row patch tensor, the bias folds in as a 13th all-ones row,
and the device kernel is one memory-bound matmul per core:
  [32768, 13] @ [13, 192]   (8-way row-sharded over B*N)

Device design (v5, emb-major / W-stationary):
  - The bottleneck is PSUM evacuation: only DVE and ACT can read PSUM
    (~1 elem/cycle/partition each; GpSimd has no PSUM port, DMA has no
    PSUM route), and each copy instruction pays a flat PSUM/SBUF access
    penalty. So the goal is full 512-wide PSUM banks and long contiguous
    copies.
  - Stationary = W columns ([13,128] for emb 0:128, [13,64] for emb
    128:192), moving = q [13, 512 nodes] -> each matmul fills one whole
    2KB PSUM bank with a single accumulation group.
  - The PE streams ~0.83 ns/col per row-band (MID p-state, never
    ramps), but matmuls in DIFFERENT row quadrants stream fully
    concurrently (measured: 2 bands 0.42 ns/col, 3 bands 0.28). W and
    q live at partition bases {0, 32, 64}, chunk c in band c%3, so
    consecutive matmuls always overlap and the PE (~18us) stays well
    under the copy bound (~28us).
  - Phase 2 (emb 128:192, M=64) packs two node-chunks per bank at
    output partition bases 0 and 64 so copies always span 128
    partitions.
  - PSUM is one flat [128, 8*512] tile; matmuls and 2-bank copies
    rotate through it, relying on Tile's subtile dependency tracking.
  - Copies alternate DVE (CAST) / ACT (COPY) with a slight ACT bias
    (ACT is 0.83 ns/elem vs DVE 1.04).
  - W rides in the first 192 columns of the q tensor so one DMA primes
    both W and the first node chunks; the q load ramps in chunks on the
    sync queue ahead of all output DMAs.
  - Output fp8e3 with a x4 pre-scale folded into W (host decodes).
"""

import numpy as np

from concourse import bacc, mybir, tile
import concourse.bass as bass
from concourse.bass_utils import run_bass_kernel_spmd

B, CIN, HIMG, WIMG = 4, 3, 512, 512
HG, WG = 256, 256          # grid after 2x2/stride-2 patching
N = HG * WG                # 65536 nodes per image
BN = B * N                 # 262144 total rows
EMB = 192
K = 13                     # 12 patch dims + 1 bias row
NCORES = 8
ROWS = BN // NCORES        # 32768 rows per core
FP8_SCALE = 4.0            # folded into W before the e3m4 downcast

CHUNK = 512                # node-cols per matmul == one full psum bank
NCHUNK = ROWS // CHUNK     # 64 chunks per core
WCOLS = EMB                # W header columns of each strip
NBAND = 3
BASES = (0, 32, 64)        # PE row-band per chunk: chunk c -> BASES[c % 3]
LCH = [(NCHUNK + NBAND - 1 - s) // NBAND for s in range(NBAND)]  # chunks/strip
SCOLS = WCOLS + max(LCH) * CHUNK   # 11456 columns per strip
QCOLS = NBAND * SCOLS              # dram q: [strip0 | strip1 | strip2]

NBANK = 8                  # psum banks; ring of 4 groups x 2 banks
GROUP = 2 * CHUNK          # elems per copy (2 banks)
NR1 = NCHUNK // 2          # 32 phase-1 rounds (2 chunks/round)
NR2 = NCHUNK // 4          # 16 phase-2 rounds (4 chunks/round)
NROUND = NR1 + NR2         # 48 rounds; round r -> banks (2*(r%4), +1)
OCOLS = NROUND * GROUP     # 49152 output cols
SGRP = 2                   # rounds per staging tile / output DMA

# input ramp, per strip: SYNC_RAMP levels on the sync queue (earliest),
# GP_RAMP levels on the gpsimd SWDGE queue (runs in parallel). Each
# dispatch costs 0.65-1.15us of serial queue time.
SYNC_RAMP = [WCOLS + 512, 1024]
GP_RAMP = [2048, 4096, 3584]
assert sum(SYNC_RAMP) + sum(GP_RAMP) == SCOLS

# ARR[c] = (strip, local) slot of node-chunk c, ordered by predicted
# DMA-arrival time so consumption order == arrival order (the Tile
# scheduler orders matmuls by modeled arrival; any mismatch stalls the
# in-order copy pipeline). Sync dispatches land ~0.8us apart from ~8.3;
# gpsimd dispatches run on a parallel queue from ~9.8, ~1.15us apart.
def _arrival_order():
    events = []  # (time, [(strip, local), ...])
    t = 8.3
    for li, csz in enumerate(SYNC_RAMP):
        base_l = sum(SYNC_RAMP[:li]) - WCOLS
        locals_ = range(max(0, base_l // CHUNK),
                        (base_l + csz) // CHUNK if li else 1)
        for s in range(NBAND):
            events.append((t, [(s, l) for l in locals_ if l < LCH[s]]))
            t += 0.8
    t = 9.8
    off = sum(SYNC_RAMP) - WCOLS
    for csz in GP_RAMP:
        lo, hi = off // CHUNK, (off + csz) // CHUNK
        for s in range(NBAND):
            events.append((t, [(s, l) for l in range(lo, hi) if l < LCH[s]]))
            t += 1.15
        off += csz
    events.sort(key=lambda e: e[0])
    return [sl for _, sls in events for sl in sls]

ARR = _arrival_order()
assert len(ARR) == NCHUNK and len(set(ARR)) == NCHUNK

# copy-engine assignment per round: 0 -> DVE, 1 -> ACT. ACT is slightly
# faster per element, so it takes 25 of 48.
ENG_OF = [(1 if r % 2 else 0) for r in range(NROUND)]
ENG_OF[24] = 1

_NC_CACHE = {}


def _build_nc(out_bufs=8, eng_of=None):
    eng_of = list(eng_of) if eng_of is not None else list(ENG_OF)
    key = (out_bufs, tuple(eng_of))
    if key in _NC_CACHE:
        return _NC_CACHE[key]
    nc = bacc.Bacc(
        "TRN2",
        target_bir_lowering=False,
        debug=False,
        enable_asserts=False,
        num_devices=NCORES,
        enable_partition_id=False,
    )
    f16 = mybir.dt.float16
    f32 = mybir.dt.float32
    f8 = mybir.dt.float8e3
    q = nc.dram_tensor("q", [K, NBAND * SCOLS], f16, kind="ExternalInput").ap()
    o8 = nc.dram_tensor("o8", [128, OCOLS], f8, kind="ExternalOutput").ap()

    with tile.TileContext(nc) as tc:
        with (
            tc.tile_pool(name="qp", bufs=1) as qpool,
            tc.tile_pool(name="ps", bufs=1, space=bass.MemorySpace.PSUM) as pspool,
            tc.tile_pool(name="ot", bufs=out_bufs) as opool,
        ):
            qw = qpool.tile([BASES[-1] + K, SCOLS], f16)
            # strip s lives at partition base BASES[s]; dispatch size-major
            # (all strips' level k before level k+1)
            offs = [0] * NBAND
            for ramp, eng in ((SYNC_RAMP, nc.sync), (GP_RAMP, nc.gpsimd)):
                for csz in ramp:
                    for s, base in enumerate(BASES):
                        off = offs[s]
                        eng.dma_start(
                            out=qw[base:base + K, off:off + csz],
                            in_=q[:, s * SCOLS + off:s * SCOLS + off + csz])
                        offs[s] += csz

            ps = pspool.tile([128, NBANK * CHUNK], f32)

            def mov(c):
                """moving AP for node-chunk c, slot ARR[c] = (strip, local)."""
                s, l = ARR[c]
                lo = WCOLS + l * CHUNK
                return qw[BASES[s]:BASES[s] + K, lo:lo + CHUNK]

            def w1(c):
                b = BASES[ARR[c][0]]
                return qw[b:b + K, 0:128]

            def w2(c):
                b = BASES[ARR[c][0]]
                return qw[b:b + K, 128:EMB]

            ssched = [SGRP] * ((NROUND - 2) // SGRP) + [1, 1]
            s0 = 0
            for snr in ssched:
                ot = opool.tile([128, snr * GROUP], f8)
                for r in range(s0, s0 + snr):
                    poff = (r % 4) * GROUP
                    if r < NR1:
                        for kk in range(2):
                            c = 2 * r + kk
                            nc.tensor.matmul(
                                ps[:, poff + kk * CHUNK:poff + (kk + 1) * CHUNK],
                                w1(c), mov(c), start=True, stop=True,
                            )
                    else:
                        for kk in range(2):
                            c0 = 4 * (r - NR1) + 2 * kk
                            dst = ps[:, poff + kk * CHUNK:poff + (kk + 1) * CHUNK]
                            nc.tensor.matmul(
                                dst[0:64, :], w2(c0), mov(c0),
                                start=True, stop=True,
                            )
                            nc.tensor.matmul(
                                dst[64:128, :], w2(c0 + 1), mov(c0 + 1),
                                start=True, stop=True,
                            )
                    src = ps[:, poff:poff + GROUP]
                    dst = ot[:, (r - s0) * GROUP:(r - s0 + 1) * GROUP]
                    if r < 2 or r == NROUND - 1:
                        # pipeline head/tail: split the round across both
                        # engines (1-bank copies) to start/drain faster
                        nc.vector.tensor_copy(dst[:, 0:CHUNK], src[:, 0:CHUNK])
                        nc.scalar.copy(dst[:, CHUNK:GROUP], src[:, CHUNK:GROUP])
                    elif eng_of[r]:
                        nc.scalar.copy(dst, src)
                    else:
                        nc.vector.tensor_copy(dst, src)
                nc.sync.dma_start(
                    out=o8[:, s0 * GROUP:(s0 + snr) * GROUP], in_=ot[:])
                s0 += snr
    nc.compile()
    _NC_CACHE[key] = nc
    return nc


def _host_prep(x, conv_w, gcn_w, gcn_b):
    x = np.asarray(x, dtype=np.float32)
    conv_w = np.asarray(conv_w, dtype=np.float32)
    gcn_w = np.asarray(gcn_w, dtype=np.float32)
    gcn_b = np.asarray(gcn_b, dtype=np.float32)

    # patches P[b, k, n]: k = (cin, ki, kj), n = r*WG + c
    P = np.ascontiguousarray(
        x.reshape(B, CIN, HG, 2, WG, 2).transpose(0, 1, 3, 5, 2, 4)
    ).reshape(B, 12, N)

    # degrees with self-loops; grid edges exist only for batch 0
    nbr = np.full((HG, WG), 4.0, np.float32)
    nbr[0, :] -= 1; nbr[-1, :] -= 1; nbr[:, 0] -= 1; nbr[:, -1] -= 1
    deg = nbr + 1.0
    deg[HG - 2, WG - 2] += 1.0          # the module's trailing extra edge
    dr = (1.0 / np.sqrt(deg)).ravel()    # dinv per node

    # batch-0 aggregation applied to the patch rows (commutes with the matmul)
    z = (dr[None, :] * P[0]).reshape(12, HG, WG)
    s = z.copy()                          # self-loop term
    s[:, 1:, :] += z[:, :-1, :]
    s[:, :-1, :] += z[:, 1:, :]
    s[:, :, 1:] += z[:, :, :-1]
    s[:, :, :-1] += z[:, :, 1:]
    s[:, HG - 2, WG - 2] += z[:, HG - 1, WG - 1]
    Q0 = dr[None, :] * s.reshape(12, N)

    Q = np.empty((K, BN), np.float32)
    Q[:12, :N] = Q0
    Q[:12, N:] = P[1:].transpose(1, 0, 2).reshape(12, 3 * N)
    Q[12, :] = 1.0                        # bias row

    Wcomb = (conv_w.reshape(EMB, 12).astype(np.float64).T
             @ gcn_w.astype(np.float64)).astype(np.float32)
    Wfull = np.concatenate([Wcomb, gcn_b[None, :]], axis=0)  # (13, 192)
    return Q, Wfull


def _decode_core(o8core, inv):
    """[128, OCOLS] fp8 -> [ROWS, EMB] fp32."""
    o = o8core.astype(np.float32) * inv
    out = np.empty((ROWS, EMB), np.float32)
    # phase 1: cols 0:ROWS, partition p = emb p, col c*512+j = node
    out[:, 0:128] = o[:, :ROWS].reshape(128, NCHUNK, CHUNK) \
        .transpose(1, 2, 0).reshape(ROWS, 128)
    # phase 2: 32 chunks of 512; partitions 0:64 = even node chunks,
    # 64:128 = odd node chunks, emb 128+p
    o2 = o[:, ROWS:].reshape(128, NCHUNK // 2, CHUNK)
    v = out[:, 128:EMB].reshape(NCHUNK // 2, 2, CHUNK, 64)
    v[:, 0] = o2[0:64].transpose(1, 2, 0)
    v[:, 1] = o2[64:128].transpose(1, 2, 0)
    return out


def kernel(x, conv_w, gcn_w, gcn_b, _trace=False, _nc_kwargs=None):
    Q, Wfull = _host_prep(x, conv_w, gcn_w, gcn_b)
    nc = _build_nc(**(_nc_kwargs or {}))
    W16 = (Wfull * FP8_SCALE).astype(np.float16)
    Q16 = Q.astype(np.float16)
    in_maps = []
    for c in range(NCORES):
        qc = Q16[:, c * ROWS:(c + 1) * ROWS].reshape(K, NCHUNK, CHUNK)
        qd = np.zeros((K, NBAND, SCOLS), np.float16)
        qd[:, :, :WCOLS] = W16[:, None, :]
        for ch in range(NCHUNK):
            s, l = ARR[ch]
            qd[:, s, WCOLS + l * CHUNK:WCOLS + (l + 1) * CHUNK] = qc[:, ch]
        in_maps.append({"q": qd.reshape(K, NBAND * SCOLS)})
    res = run_bass_kernel_spmd(nc, in_maps, list(range(NCORES)), trace=_trace)
    inv = np.float32(1.0 / FP8_SCALE)
    out = np.empty((NCORES, ROWS, EMB), np.float32)
    for c in range(NCORES):
        out[c] = _decode_core(res.results[c]["o8"], inv)
    out = out.reshape(B, N, EMB)
    if _trace:
        return out, res
    return out
